# revision 25
# baseline (speedup 1.0000x reference)
"""Trainium2 Bass kernel for padded/ragged multi-head attention.

Problem shape (hardcoded, matches the grading harness):
  B=8 sequences, S=1024 padded length, VALID=512 valid tokens/seq,
  H=1024 hidden, NH=16 heads, HD=64 head dim, T=B*VALID=4096 tokens.

Sharding: pure data parallel, one batch per NeuronCore (8 cores).

Because the valid tokens of each sequence sit at positions [0, VALID) and all
padded key positions carry a -1e9 additive bias (exp underflows to exactly 0
in fp32), the padded-attention computation reduces exactly to dense attention
over each sequence's 512 valid tokens.  Padding is never materialized.

Per-core pipeline (feature-major, zero transposes):
  1. Q^T/K^T = W_qk^T X^T (features on partitions), interleaved q,k so head
     pairs complete early.  First x/weight chunks are split small so the
     first matmul starts as soon as ~160KB has landed.
  2. RoPE: roped = q*cos + R(q)*sin with R a +-1 signed-permutation matmul.
  3. scores^T per head pair via PE row-tiling: each head's K half (64
     contraction rows) runs in its own row-group, two heads concurrently --
     no zero-padded K tiles and half the PE slots.  The scalar-engine exp
     stream (the pacing engine of this phase) overlaps everything.
  4. V projection chunks interleaved into the same phase: they don't touch
     the exp-gated PSUM pools, so the PE fills the slack the row-tiled
     scores left while Scalar drains the exp backlog.  wv/wo weight loads
     are chunked between the wqk tile loads on the sync ring.
  5. ctx matmuls (token-major V with an appended ones column -> ctx^T and
     the softmax denominator in one PSUM tile), normalized per head via one
     DVE reciprocal_approx_fast on the PSUM denominator row + GpSimd
     partition-broadcast + one DVE multiply.
  6. o_proj pass n=0 is software-pipelined into the ctx loop (column m-2
     emitted after ctx pair m), pass n=1 follows; accumulation groups
     interleave across PSUM banks.

Matmul operands are fp16 (1 cycle/row on the PE, fast weight load, fp32 PSUM
accumulation; end-to-end relative error ~6e-4).
"""

import sys
import numpy as np

sys.path.insert(0, "/opt/trn_rl_repo")


def _ensure_ntff_hook():
    """The container's stub `antenv` lacks `axon_hooks`; run_bass_kernel_spmd
    imports it when tracing is requested (e.g. BASS_TRACE=1).  Register a
    functional shim backed by libaxon_pjrt's profiling symbols so a tracing
    harness doesn't crash."""
    import types
    try:
        import antenv
    except ImportError:
        return
    if "antenv.axon_hooks" in sys.modules:
        return
    mod = types.ModuleType("antenv.axon_hooks")
    state = {"hook": None}
    mod.set_axon_ntff_profile_hook = lambda h: state.__setitem__("hook", h)
    mod.get_axon_ntff_profile_hook = lambda: state["hook"]
    sys.modules["antenv.axon_hooks"] = mod
    antenv.axon_hooks = mod
    try:
        if "/root/.axon_site" not in sys.path:
            sys.path.insert(0, "/root/.axon_site")
        from trn_agent_boot.trn_boot import _ntff_profile_via_ctypes
        mod.set_axon_ntff_profile_hook(
            _ntff_profile_via_ctypes("/opt/axon/libaxon_pjrt.so"))
    except Exception:
        pass


_ensure_ntff_hook()

B = 8
S = 1024
H = 1024
NH = 16
HD = 64
VALID = 512
P = 128
KC = H // P            # 8 contraction chunks of 128
QK_TILES = 2 * H // P  # 16 feature-major tiles for Q^T and K^T
TC = VALID // P        # 4 token chunks
VW = HD + 1            # V columns per head incl. ones column

_CACHE = {}


def _build(with_qkv_bias):
    import concourse.mybir as mybir
    import concourse.tile as tile
    from concourse import bacc
    from contextlib import ExitStack

    F32 = mybir.dt.float32
    F16 = mybir.dt.float16
    EXP = mybir.ActivationFunctionType.Exp

    nc = bacc.Bacc()
    xT = nc.declare_dram_parameter("xT", [P, KC, VALID], F16, isOutput=False)
    wqk = nc.declare_dram_parameter("wqk", [QK_TILES, P, KC, P], F16, isOutput=False)
    wv = nc.declare_dram_parameter("wv", [2, P, KC, 512], F16, isOutput=False)
    wo = nc.declare_dram_parameter("wo", [2, P, KC, 512], F16, isOutput=False)
    cos2 = nc.declare_dram_parameter("cos2", [P, VALID], F16, isOutput=False)
    sin2 = nc.declare_dram_parameter("sin2", [P, VALID], F16, isOutput=False)
    rot = nc.declare_dram_parameter("rot", [P, P], F16, isOutput=False)
    biask = nc.declare_dram_parameter("biask", [P, TC], F32, isOutput=False)
    ones16 = nc.declare_dram_parameter("ones16", [P, 64], F16, isOutput=False)
    if with_qkv_bias:
        qb_rope = nc.declare_dram_parameter("qb_rope", [P, QK_TILES, VALID], F32, isOutput=False)
        vbias = nc.declare_dram_parameter("vbias", [P, KC], F32, isOutput=False)
    out = nc.declare_dram_parameter("out", [VALID, H], F32, isOutput=True)

    with tile.TileContext(nc) as tc:
        with ExitStack() as ctx:
            consts = ctx.enter_context(tc.tile_pool(name="consts", bufs=1))
            xpool = ctx.enter_context(tc.tile_pool(name="x", bufs=1))
            # per-m weight tiles, loaded in consumption order, deep prefetch
            wqk_pool = ctx.enter_context(tc.tile_pool(name="wqk", bufs=8))
            wno_pool = ctx.enter_context(tc.tile_pool(name="wno", bufs=4))
            qsb_pool = ctx.enter_context(tc.tile_pool(name="qsb", bufs=3))
            qk_pool = ctx.enter_context(tc.tile_pool(name="qk", bufs=NH))
            v_pool = ctx.enter_context(tc.tile_pool(name="v", bufs=TC))
            e_pool = ctx.enter_context(tc.tile_pool(name="e", bufs=NH * TC // 2))
            ctx_pool = ctx.enter_context(tc.tile_pool(name="ctx", bufs=KC))
            tmp_pool = ctx.enter_context(tc.tile_pool(name="tmp", bufs=6))
            lg_pool = ctx.enter_context(tc.tile_pool(name="lg", bufs=2))
            rc_pool = ctx.enter_context(tc.tile_pool(name="rc", bufs=KC))
            o_pool = ctx.enter_context(tc.tile_pool(name="o", bufs=3))
            proj_ps = ctx.enter_context(tc.tile_pool(name="pps", bufs=2, space="PSUM"))
            rot_ps = ctx.enter_context(tc.tile_pool(name="rps", bufs=1, space="PSUM"))
            s_ps = ctx.enter_context(tc.tile_pool(name="sps", bufs=2, space="PSUM"))
            den_ps = ctx.enter_context(tc.tile_pool(name="dps", bufs=1, space="PSUM"))

            # PE warm-up: the HAM clock gate needs ~3.4us of sustained
            # matmul activity to lift the PE from 1.2 to 2.4 GHz.  Run dummy
            # matmuls on a memset scratch tile while the first input DMAs
            # are still in flight; the first real matmul then starts warm.
            warm = consts.tile([P, VALID], F16, tag="warm")
            nc.gpsimd.memset(warm[:], 0.0)
            wps = proj_ps.tile([P, VALID], F32, tag="pps", name="wps")
            for r in range(8):
                nc.tensor.matmul(wps[:], warm[:, 0:P], warm[:],
                                 start=True, stop=True, skip_group_check=True)

            # Each HWDGE trigger costs ~0.65us on its sequencer, serialized
            # per ring.  Split x/weights so the first matmul chain's inputs
            # (x kc0 + wqk[0] kc0-1, ~160KB) land first; the chain start
            # hides the remaining transfers.
            xa0 = xpool.tile([P, 1, VALID], F16, tag="xa0", name="xa0")
            nc.scalar.dma_start(xa0[:], xT[:, 0:1, :])
            xa1 = xpool.tile([P, 1, VALID], F16, tag="xa1", name="xa1")
            nc.scalar.dma_start(xa1[:], xT[:, 1:2, :])
            xa2 = xpool.tile([P, 2, VALID], F16, tag="xa2", name="xa2")
            nc.scalar.dma_start(xa2[:], xT[:, 2:4, :])

            wm_tiles = {}
            wm0 = wqk_pool.tile([P, KC, P], F16, tag="wqk", name="wm0")
            nc.sync.dma_start(wm0[:, 0:2, :], wqk[0][:, 0:2, :])
            nc.sync.dma_start(wm0[:, 2:8, :], wqk[0][:, 2:8, :])
            wm_tiles[0] = wm0
            xb = xpool.tile([P, 4, VALID], F16, tag="xb", name="xb")
            nc.sync.dma_start(xb[:], xT[:, 4:8, :])

            # small consts on the scalar HWDGE ring, after x
            cos_t = consts.tile([P, VALID], F16, tag="cos")
            sin_t = consts.tile([P, VALID], F16, tag="sin")
            rot_t = consts.tile([P, P], F16, tag="rot")
            bias_t = consts.tile([P, TC], F32, tag="biask")
            ones_t = consts.tile([P, 64], F16, tag="ones16")
            nc.scalar.dma_start(cos_t[:], cos2[:])
            nc.scalar.dma_start(sin_t[:], sin2[:])
            nc.scalar.dma_start(rot_t[:], rot[:])
            nc.scalar.dma_start(bias_t[:], biask[:])
            nc.scalar.dma_start(ones_t[:], ones16[:])
            if with_qkv_bias:
                qb_t = consts.tile([P, QK_TILES, VALID], F32, tag="qb")
                nc.scalar.dma_start(qb_t[:], qb_rope[:])
                vb_t = consts.tile([P, KC], F32, tag="vb")
                nc.scalar.dma_start(vb_t[:], vbias[:])

            # pin the activation table to natural_log_exp_and_others (set 6:
            # exp, ln, copy, identity all present); placed after the initial
            # DMA triggers so it doesn't delay the first transfers
            nc.scalar.add_instruction(
                mybir.InstLoadActFuncSet(
                    name=nc.get_next_instruction_name(), ins=[], outs=[],
                    act_func_set_id=6,
                )
            )

            def x_sl(kc):
                if kc < 1:
                    return xa0[:, 0, :]
                if kc < 2:
                    return xa1[:, 0, :]
                if kc < 4:
                    return xa2[:, kc - 2, :]
                return xb[:, kc - 4, :]

            escale = 1.0 / np.sqrt(HD)
            qk_tiles = {}
            k_tiles = {}
            e_tiles = {}

            # V/O weight tiles; transfers are chunked between the wqk tile
            # loads on the sync ring so the qk weight stream never starves.
            wvns = [wno_pool.tile([P, KC, 512], F16, tag="wno", name=f"wv{n}")
                    for n in range(2)]
            wons = [wno_pool.tile([P, KC, 512], F16, tag="wno", name=f"wo{n}")
                    for n in range(2)]

            def emit_wchunk(ci):
                # 16 chunks of (P, 2, 512): wv0, wv1 then wo0, wo1
                src, dst = [(wv, wvns), (wo, wons)][ci // 8]
                n = (ci // 4) % 2
                kc = 2 * (ci % 4)
                nc.sync.dma_start(dst[n][:, kc : kc + 2, :], src[n][:, kc : kc + 2, :])

            # token-major V tiles, one 512-wide half per qkv weight half
            v_tiles = [v_pool.tile([P, 2, 512], F16, tag="v", name=f"v{t}") for t in range(TC)]

            def emit_rot(pend):
                q_sb, m = pend
                rp = rot_ps.tile([P, VALID], F32, tag="rps", name=f"rp{m}")
                nc.tensor.matmul(rp[:], rot_t[:], q_sb[:], start=True, stop=True)
                t1 = tmp_pool.tile([P, VALID], F16, tag="t1", name=f"t1_{m}")
                nc.gpsimd.tensor_mul(t1[:], q_sb[:], cos_t[:])
                t2 = tmp_pool.tile([P, VALID], F16, tag="t2", name=f"t2_{m}")
                nc.vector.tensor_mul(t2[:], rp[:], sin_t[:])
                dst = qk_pool.tile([P, VALID], F16, tag="qk", name=f"qk{m}")
                if with_qkv_bias:
                    t3 = tmp_pool.tile([P, VALID], F32, tag="t3", name=f"t3_{m}")
                    nc.vector.tensor_add(t3[:], t1[:], t2[:])
                    nc.vector.tensor_add(dst[:], t3[:], qb_t[:, m, :])
                else:
                    nc.vector.tensor_add(dst[:], t1[:], t2[:])
                if m >= NH // 2:
                    k_tiles[m - NH // 2] = dst
                else:
                    qk_tiles[m] = dst

            def emit_scores(pair):
                # PE row-tiling: head 2*pair lives on contraction rows 0:64,
                # head 2*pair+1 on rows 64:128.  Both heads' scores for a key
                # chunk land in the two banks of one (128, 1024) PSUM tile --
                # the matmuls run concurrently (different row groups AND
                # banks) and a single exp covers both, halving the
                # scalar-engine per-activation overhead.  The per-key bias
                # is shared by both heads, so this is always exact.
                qt = qk_tiles[pair]
                kt = k_tiles[pair]
                for j in range(TC):
                    sp = s_ps.tile([P, 2 * VALID], F32, tag="sps",
                                   name=f"s{pair}_{j}")
                    for hh in range(2):
                        lo, hi = hh * HD, hh * HD + HD
                        nc.tensor.matmul(
                            sp[:, hh * VALID : (hh + 1) * VALID],
                            kt[lo:hi, j * P : (j + 1) * P],
                            qt[lo:hi, :],
                            start=True, stop=True,
                        )
                    ej = e_pool.tile([P, 2 * VALID], F16, tag="e",
                                     name=f"e{pair}_{j}")
                    nc.scalar.activation(
                        ej[:], sp[:], EXP, bias=bias_t[:, j : j + 1], scale=escale
                    )
                    e_tiles[(2 * pair, j)] = ej[:, 0:VALID]
                    e_tiles[(2 * pair + 1, j)] = ej[:, VALID : 2 * VALID]

            def emit_vproj(n, t):
                ps = proj_ps.tile([P, 512], F32, tag="pps", name=f"vps{n}{t}")
                for kc in range(KC):
                    nc.tensor.matmul(
                        ps[:], x_sl(kc)[:, t * P : (t + 1) * P], wvns[n][:, kc, :],
                        start=(kc == 0), stop=(kc == KC - 1),
                    )
                nc.vector.tensor_copy(v_tiles[t][:, n, :], ps[:])

            rcs = {}

            def emit_den(p):
                # softmax denominators for head pair p via a col-tiled
                # ones-matmul: each head's column sums land replicated on a
                # 64-partition half of one PSUM tile, matching the ctx^T
                # halves, so the normalize multiply reads the batched
                # reciprocal directly -- no cross-partition broadcast.
                dt_ = den_ps.tile([P, VALID], F32, tag="dps", name=f"den{p}")
                for j in range(TC):
                    for hh in range(2):
                        nc.tensor.matmul(
                            dt_[hh * HD : (hh + 1) * HD, :], ones_t[:, 0:HD],
                            e_tiles[(2 * p + hh, j)],
                            start=(j == 0), stop=(j == TC - 1),
                            skip_group_check=True,
                        )
                lg = lg_pool.tile([P, VALID], F32, tag="lg", name=f"lg{p}")
                nc.scalar.activation(lg[:], dt_[:],
                                     mybir.ActivationFunctionType.Ln)
                rc = rc_pool.tile([P, VALID], F16, tag="rc", name=f"rc{p}")
                nc.scalar.activation(rc[:], lg[:], EXP, scale=-1.0)
                rcs[p] = rc

            # ---- Phase B+C: QK proj + RoPE + scores/exp + V proj, fused ----
            # V-proj chunks fill the PE slack that row-tiled scores free
            # while the scalar engine drains the exp backlog.
            order = [m for pair in range(NH // 2) for m in (pair, NH // 2 + pair)]
            v_sched = {4: (0, 0), 5: (0, 1), 6: (0, 2), 7: (0, 3),
                       9: (1, 0), 10: (1, 1), 11: (1, 2), 12: (1, 3)}
            pend = None
            for mi, m in enumerate(order):
                if m in wm_tiles:
                    wm = wm_tiles[m]
                else:
                    wm = wqk_pool.tile([P, KC, P], F16, tag="wqk", name=f"wm{m}")
                    nc.sync.dma_start(wm[:], wqk[m])
                if 1 <= mi <= 16:
                    emit_wchunk(mi - 1)
                ps = proj_ps.tile([P, VALID], F32, tag="pps", name=f"ps{m}")
                for kc in range(KC):
                    nc.tensor.matmul(
                        ps[:], wm[:, kc, :], x_sl(kc),
                        start=(kc == 0), stop=(kc == KC - 1),
                    )
                q_sb = qsb_pool.tile([P, VALID], F16, tag="qsb", name=f"qsb{m}")
                nc.vector.tensor_copy(q_sb[:], ps[:])
                if pend is not None:
                    pm = pend[1]
                    emit_rot(pend)
                    if pm >= NH // 2:
                        emit_scores(pm - NH // 2)
                if mi in v_sched:
                    emit_vproj(*v_sched[mi])
                if mi >= 4 and mi % 2 == 0:
                    emit_den((mi - 4) // 2)
                pend = (q_sb, m)
            emit_rot(pend)
            emit_scores(NH // 2 - 1)
            emit_wchunk(15)
            emit_den(6)
            emit_den(7)

            # ---- Phase D+E: ctx + normalize, o_proj pass 0 pipelined in ----
            # ctx pairs are col-tiled (2 heads concurrently into the two
            # 64-partition halves of one PSUM bank); the reciprocals were
            # precomputed during phase B+C, so a pair's bank is freed by
            # two DVE multiplies -- no scalar/gpsimd latency in the loop.
            ctx_tiles = [ctx_pool.tile([P, VALID], F16, tag="ctx", name=f"ctx{m}") for m in range(KC)]

            # o_proj pass n=0 accumulation chains on the four single-bank
            # pools; fed column-by-column with a 3-pair stagger.
            ops0 = [
                proj_ps.tile([P, 512], F32, tag="pps", name="od00"),
                proj_ps.tile([P, 512], F32, tag="pps", name="od01"),
                rot_ps.tile([P, 512], F32, tag="rps", name="od02"),
                den_ps.tile([P, 512], F32, tag="dps", name="od03"),
            ]

            def emit_ocol(mcol, n, chains):
                won = wons[n]
                for t in range(TC):
                    nc.tensor.matmul(
                        chains[t][:], ctx_tiles[mcol][:, t * P : (t + 1) * P],
                        won[:, mcol, :],
                        start=(mcol == 0), stop=(mcol == KC - 1),
                        skip_group_check=True,
                    )

            cpd = None
            for p in range(KC):
                if p % 2 == 0:
                    cpd = s_ps.tile([P, 2 * VALID], F32, tag="sps", name=f"cpd{p//2}")
                cp = cpd[:, (p % 2) * VALID : (p % 2 + 1) * VALID]
                vn, vcol = p // 4, (p % 4) * P
                for j in range(TC):
                    for hh in range(2):
                        h = 2 * p + hh
                        nc.tensor.matmul(
                            cp[hh * HD : (hh + 1) * HD, :],
                            v_tiles[j][:, vn, vcol + hh * HD : vcol + (hh + 1) * HD],
                            e_tiles[(h, j)],
                            start=(j == 0), stop=(j == TC - 1),
                            skip_group_check=True,
                        )
                for hh in range(2):
                    h = 2 * p + hh
                    dst = ctx_tiles[p][hh * HD : (hh + 1) * HD, :]
                    rch = rcs[p][hh * HD : (hh + 1) * HD, :]
                    if with_qkv_bias:
                        tmpc = tmp_pool.tile([HD, VALID], F32, tag="tc", name=f"tc{h}")
                        nc.vector.tensor_mul(tmpc[:], cp[hh * HD : (hh + 1) * HD, :], rch)
                        nc.scalar.activation(
                            dst, tmpc[:], mybir.ActivationFunctionType.Identity,
                            bias=vb_t[hh * HD : (hh + 1) * HD, p : p + 1],
                        )
                    else:
                        nc.vector.tensor_mul(dst, cp[hh * HD : (hh + 1) * HD, :], rch)
                if p >= 3:
                    emit_ocol(p - 3, 0, ops0)
            for mcol in (KC - 3, KC - 2, KC - 1):
                emit_ocol(mcol, 0, ops0)
            for t in range(TC):
                ot = o_pool.tile([P, 512], F32, tag="o", name=f"o0{t}")
                nc.vector.tensor_copy(ot[:], ops0[t][:])
                nc.sync.dma_start(out[t * P : (t + 1) * P, 0:512], ot[:])

            # o_proj pass n=1: all ctx tiles are ready; straight chains on
            # the same four single-bank pools (a shared double-tile would
            # serialize chain t+1 behind chain t's drain copy -- Tile
            # tracks write-after-read at tile granularity)
            ops1 = [
                proj_ps.tile([P, 512], F32, tag="pps", name="od10"),
                proj_ps.tile([P, 512], F32, tag="pps", name="od11"),
                rot_ps.tile([P, 512], F32, tag="rps", name="od12"),
                den_ps.tile([P, 512], F32, tag="dps", name="od13"),
            ]
            for t in range(TC):
                ps = ops1[t]
                for m in range(KC):
                    nc.tensor.matmul(
                        ps[:], ctx_tiles[m][:, t * P : (t + 1) * P], wons[1][:, m, :],
                        start=(m == 0), stop=(m == KC - 1),
                        skip_group_check=True,
                    )
                ot = o_pool.tile([P, 512], F32, tag="o", name=f"o1{t}")
                nc.vector.tensor_copy(ot[:], ps[:])
                nc.sync.dma_start(out[t * P : (t + 1) * P, 512:1024], ot[:])

    nc.compile()
    return nc


def _get_nc(with_qkv_bias):
    key = bool(with_qkv_bias)
    if key not in _CACHE:
        _CACHE[key] = _build(key)
    return _CACHE[key]


def _rot_matrix():
    # R such that (R.T @ q)[d] == rotate_half(q)[d], block-diagonal per head
    r64 = np.zeros((HD, HD), np.float32)
    half = HD // 2
    for d in range(half):
        r64[d + half, d] = -1.0  # dest d < 32 gets -q[d+32]
        r64[d, d + half] = 1.0   # dest d >= 32 gets  q[d-32]
    r = np.zeros((P, P), np.float32)
    r[:HD, :HD] = r64
    r[HD:, HD:] = r64
    return r


def _to_tiles_kxm(w, ncols):
    """(H, F) weight -> (F//ncols, P, KC, ncols) fp16, contiguous."""
    F = w.shape[1]
    t = w.reshape(KC, P, F // ncols, ncols).transpose(2, 1, 0, 3)
    return np.ascontiguousarray(t.astype(np.float16))


def kernel(hidden_states, cos, sin, attention_bias, qkv_w, qkv_b, o_w, o_b,
           indices, batch, seqlen, _trace=False):
    from concourse.bass_utils import run_bass_kernel_spmd

    hidden_states = np.asarray(hidden_states, dtype=np.float32)
    cos = np.asarray(cos, dtype=np.float32)
    sin = np.asarray(sin, dtype=np.float32)
    attention_bias = np.asarray(attention_bias, dtype=np.float32)
    qkv_w = np.asarray(qkv_w, dtype=np.float32)
    qkv_b = np.asarray(qkv_b, dtype=np.float32)
    o_w = np.asarray(o_w, dtype=np.float32)
    o_b = np.asarray(o_b, dtype=np.float32)
    indices = np.asarray(indices)
    batch = int(batch)
    seqlen = int(seqlen)

    with_bias = bool(np.any(qkv_b))

    pos = indices.astype(np.int64)
    b_of = pos // seqlen
    s_of = pos % seqlen

    wqk2 = _to_tiles_kxm(qkv_w[:, : 2 * H], P)        # (16, P, KC, P)
    wv2 = _to_tiles_kxm(qkv_w[:, 2 * H :], 512)       # (2, P, KC, 512)
    wo2 = _to_tiles_kxm(o_w, 512)                     # (2, P, KC, 512)
    rot = _rot_matrix().astype(np.float16)
    ones16 = np.ones((P, 64), np.float16)

    in_maps = []
    tok_idx = []
    for b in range(batch):
        idx = np.nonzero(b_of == b)[0]
        assert len(idx) == VALID, f"batch {b} has {len(idx)} valid tokens"
        tok_idx.append(idx)
        xT2 = np.ascontiguousarray(
            hidden_states[idx].T.reshape(KC, P, VALID).transpose(1, 0, 2)
            .astype(np.float16)
        )
        cosT = cos[idx, 0, :].T  # (HD, VALID)
        sinT = sin[idx, 0, :].T
        cos2 = np.ascontiguousarray(
            np.concatenate([cosT, cosT], axis=0).astype(np.float16))
        sin2 = np.ascontiguousarray(
            np.concatenate([sinT, sinT], axis=0).astype(np.float16))
        bias_b = attention_bias[b, 0, 0, s_of[idx]].astype(np.float32)  # (VALID,)
        biask = np.ascontiguousarray(bias_b.reshape(TC, P).T)  # (P, TC)
        m = {
            "xT": xT2, "wqk": wqk2, "wv": wv2, "wo": wo2,
            "cos2": cos2, "sin2": sin2, "rot": rot, "biask": biask,
            "ones16": ones16,
        }
        if with_bias:
            bq = qkv_b[: 2 * H]
            cos_full = np.tile(cosT, (2 * H // HD, 1))  # (2H, VALID)
            sin_full = np.tile(sinT, (2 * H // HD, 1))
            rot_bq = bq.reshape(-1, 2, HD // 2)[:, ::-1, :].copy()
            rot_bq[:, 0, :] *= -1.0
            rot_bq = rot_bq.reshape(-1)
            qb = (bq[:, None] * cos_full + rot_bq[:, None] * sin_full)
            qb = qb.reshape(QK_TILES, P, VALID).transpose(1, 0, 2)
            m["qb_rope"] = np.ascontiguousarray(qb.astype(np.float32))
            bv = qkv_b[2 * H :].astype(np.float32)
            m["vbias"] = np.ascontiguousarray(bv.reshape(KC, P).T)
        in_maps.append(m)

    nc = _get_nc(with_bias)
    res = run_bass_kernel_spmd(nc, in_maps, core_ids=list(range(B)), trace=_trace)

    T = hidden_states.shape[0]
    out_full = np.empty((T, H), np.float32)
    for b in range(batch):
        out_full[tok_idx[b]] = res.results[b]["out"]
    if np.any(o_b):
        out_full += o_b[None, :]
    if _trace:
        kernel.last_exec_time_ns = res.exec_time_ns
        kernel.last_results = res
    return out_full


# revision 26
# speedup vs baseline: 1.0497x; 1.0497x over previous
"""Trainium2 Bass kernel for padded/ragged multi-head attention.

Problem shape (hardcoded, matches the grading harness):
  B=8 sequences, S=1024 padded length, VALID=512 valid tokens/seq,
  H=1024 hidden, NH=16 heads, HD=64 head dim, T=B*VALID=4096 tokens.

Sharding: pure data parallel, one batch per NeuronCore (8 cores).

Because the valid tokens of each sequence sit at positions [0, VALID) and all
padded key positions carry a -1e9 additive bias (exp underflows to exactly 0
in fp32), the padded-attention computation reduces exactly to dense attention
over each sequence's 512 valid tokens.  Padding is never materialized.

Per-core pipeline (feature-major, zero transposes):
  1. Q^T/K^T = W_qk^T X^T (features on partitions), interleaved q,k so head
     pairs complete early.  First x/weight chunks are split small so the
     first matmul starts as soon as ~160KB has landed.
  2. RoPE: roped = q*cos + R(q)*sin with R a +-1 signed-permutation matmul.
  3. scores^T per head pair via PE row-tiling: each head's K half (64
     contraction rows) runs in its own row-group, two heads concurrently --
     no zero-padded K tiles and half the PE slots.  The scalar-engine exp
     stream (the pacing engine of this phase) overlaps everything.
  4. V projection chunks interleaved into the same phase: they don't touch
     the exp-gated PSUM pools, so the PE fills the slack the row-tiled
     scores left while Scalar drains the exp backlog.  wv/wo weight loads
     are chunked between the wqk tile loads on the sync ring.
  5. ctx matmuls (token-major V with an appended ones column -> ctx^T and
     the softmax denominator in one PSUM tile), normalized per head via one
     DVE reciprocal_approx_fast on the PSUM denominator row + GpSimd
     partition-broadcast + one DVE multiply.
  6. o_proj pass n=0 is software-pipelined into the ctx loop (column m-2
     emitted after ctx pair m), pass n=1 follows; accumulation groups
     interleave across PSUM banks.

Matmul operands are fp16 (1 cycle/row on the PE, fast weight load, fp32 PSUM
accumulation; end-to-end relative error ~6e-4).
"""

import sys
import numpy as np

sys.path.insert(0, "/opt/trn_rl_repo")


def _ensure_ntff_hook():
    """The container's stub `antenv` lacks `axon_hooks`; run_bass_kernel_spmd
    imports it when tracing is requested (e.g. BASS_TRACE=1).  Register a
    functional shim backed by libaxon_pjrt's profiling symbols so a tracing
    harness doesn't crash."""
    import types
    try:
        import antenv
    except ImportError:
        return
    if "antenv.axon_hooks" in sys.modules:
        return
    mod = types.ModuleType("antenv.axon_hooks")
    state = {"hook": None}
    mod.set_axon_ntff_profile_hook = lambda h: state.__setitem__("hook", h)
    mod.get_axon_ntff_profile_hook = lambda: state["hook"]
    sys.modules["antenv.axon_hooks"] = mod
    antenv.axon_hooks = mod
    try:
        if "/root/.axon_site" not in sys.path:
            sys.path.insert(0, "/root/.axon_site")
        from trn_agent_boot.trn_boot import _ntff_profile_via_ctypes
        mod.set_axon_ntff_profile_hook(
            _ntff_profile_via_ctypes("/opt/axon/libaxon_pjrt.so"))
    except Exception:
        pass


_ensure_ntff_hook()

B = 8
S = 1024
H = 1024
NH = 16
HD = 64
VALID = 512
P = 128
KC = H // P            # 8 contraction chunks of 128
QK_TILES = 2 * H // P  # 16 feature-major tiles for Q^T and K^T
TC = VALID // P        # 4 token chunks
VW = HD + 1            # V columns per head incl. ones column

_CACHE = {}


def _build(with_qkv_bias):
    import concourse.mybir as mybir
    import concourse.tile as tile
    from concourse import bacc
    from contextlib import ExitStack

    F32 = mybir.dt.float32
    F16 = mybir.dt.float16
    EXP = mybir.ActivationFunctionType.Exp

    nc = bacc.Bacc()
    xT = nc.declare_dram_parameter("xT", [P, KC, VALID], F16, isOutput=False)
    wqk = nc.declare_dram_parameter("wqk", [QK_TILES, P, KC, P], F16, isOutput=False)
    wv = nc.declare_dram_parameter("wv", [2, P, KC, 512], F16, isOutput=False)
    wo = nc.declare_dram_parameter("wo", [2, P, KC, 512], F16, isOutput=False)
    cos2 = nc.declare_dram_parameter("cos2", [P, VALID], F16, isOutput=False)
    sin2 = nc.declare_dram_parameter("sin2", [P, VALID], F16, isOutput=False)
    rot = nc.declare_dram_parameter("rot", [P, P], F16, isOutput=False)
    biask = nc.declare_dram_parameter("biask", [P, TC], F32, isOutput=False)
    ones16 = nc.declare_dram_parameter("ones16", [P, 64], F16, isOutput=False)
    if with_qkv_bias:
        qb_rope = nc.declare_dram_parameter("qb_rope", [P, QK_TILES, VALID], F32, isOutput=False)
        vbias = nc.declare_dram_parameter("vbias", [P, KC], F32, isOutput=False)
    out = nc.declare_dram_parameter("out", [VALID, H], F32, isOutput=True)

    with tile.TileContext(nc) as tc:
        with ExitStack() as ctx:
            consts = ctx.enter_context(tc.tile_pool(name="consts", bufs=1))
            xpool = ctx.enter_context(tc.tile_pool(name="x", bufs=1))
            # per-m weight tiles, loaded in consumption order, deep prefetch
            wqk_pool = ctx.enter_context(tc.tile_pool(name="wqk", bufs=6))
            wno_pool = ctx.enter_context(tc.tile_pool(name="wno", bufs=4))
            qsb_pool = ctx.enter_context(tc.tile_pool(name="qsb", bufs=3))
            qk_pool = ctx.enter_context(tc.tile_pool(name="qk", bufs=NH))
            v_pool = ctx.enter_context(tc.tile_pool(name="v", bufs=TC))
            e_pool = ctx.enter_context(tc.tile_pool(name="e", bufs=NH * TC // 2))
            ctx_pool = ctx.enter_context(tc.tile_pool(name="ctx", bufs=KC))
            tmp_pool = ctx.enter_context(tc.tile_pool(name="tmp", bufs=3))
            lg_pool = ctx.enter_context(tc.tile_pool(name="lg", bufs=2))
            rc_pool = ctx.enter_context(tc.tile_pool(name="rc", bufs=KC))
            o_pool = ctx.enter_context(tc.tile_pool(name="o", bufs=3))
            proj_ps = ctx.enter_context(tc.tile_pool(name="pps", bufs=2, space="PSUM"))
            rot_ps = ctx.enter_context(tc.tile_pool(name="rps", bufs=1, space="PSUM"))
            s_ps = ctx.enter_context(tc.tile_pool(name="sps", bufs=2, space="PSUM"))
            den_ps = ctx.enter_context(tc.tile_pool(name="dps", bufs=1, space="PSUM"))

            # Each HWDGE trigger costs ~0.65us on its sequencer, serialized
            # per ring.  Split x/weights so the first matmul chain's inputs
            # (x kc0 + wqk[0] kc0-1, ~160KB) land first; the chain start
            # hides the remaining transfers.
            xa0 = xpool.tile([P, 1, VALID], F16, tag="xa0", name="xa0")
            nc.scalar.dma_start(xa0[:], xT[:, 0:1, :])
            xa1 = xpool.tile([P, 1, VALID], F16, tag="xa1", name="xa1")
            nc.scalar.dma_start(xa1[:], xT[:, 1:2, :])
            xa2 = xpool.tile([P, 2, VALID], F16, tag="xa2", name="xa2")
            nc.scalar.dma_start(xa2[:], xT[:, 2:4, :])

            wm_tiles = {}
            wm0 = wqk_pool.tile([P, KC, P], F16, tag="wqk", name="wm0")
            nc.sync.dma_start(wm0[:, 0:2, :], wqk[0][:, 0:2, :])
            nc.sync.dma_start(wm0[:, 2:8, :], wqk[0][:, 2:8, :])
            wm_tiles[0] = wm0
            xb = xpool.tile([P, 4, VALID], F16, tag="xb", name="xb")
            nc.sync.dma_start(xb[:], xT[:, 4:8, :])

            # small consts on the scalar HWDGE ring, after x
            cos_t = consts.tile([P, VALID], F16, tag="cos")
            sin_t = consts.tile([P, VALID], F16, tag="sin")
            rot_t = consts.tile([P, P], F16, tag="rot")
            bias_t = consts.tile([P, TC], F32, tag="biask")
            ones_t = consts.tile([P, 64], F16, tag="ones16")
            nc.scalar.dma_start(cos_t[:], cos2[:])
            nc.scalar.dma_start(sin_t[:], sin2[:])
            nc.scalar.dma_start(rot_t[:], rot[:])
            nc.scalar.dma_start(bias_t[:], biask[:])
            nc.scalar.dma_start(ones_t[:], ones16[:])
            if with_qkv_bias:
                qb_t = consts.tile([P, QK_TILES, VALID], F32, tag="qb")
                nc.scalar.dma_start(qb_t[:], qb_rope[:])
                vb_t = consts.tile([P, KC], F32, tag="vb")
                nc.scalar.dma_start(vb_t[:], vbias[:])

            # pin the activation table to natural_log_exp_and_others (set 6:
            # exp, ln, copy, identity all present); placed after the initial
            # DMA triggers so it doesn't delay the first transfers
            nc.scalar.add_instruction(
                mybir.InstLoadActFuncSet(
                    name=nc.get_next_instruction_name(), ins=[], outs=[],
                    act_func_set_id=6,
                )
            )

            def x_sl(kc):
                if kc < 1:
                    return xa0[:, 0, :]
                if kc < 2:
                    return xa1[:, 0, :]
                if kc < 4:
                    return xa2[:, kc - 2, :]
                return xb[:, kc - 4, :]

            escale = 1.0 / np.sqrt(HD)
            qk_tiles = {}
            k_tiles = {}
            e_tiles = {}

            # V/O weight tiles; transfers are chunked between the wqk tile
            # loads on the sync ring so the qk weight stream never starves.
            wvns = [wno_pool.tile([P, KC, 512], F16, tag="wno", name=f"wv{n}")
                    for n in range(2)]
            wons = [wno_pool.tile([P, KC, 512], F16, tag="wno", name=f"wo{n}")
                    for n in range(2)]

            def emit_wchunk(ci):
                # 16 chunks of (P, 2, 512): wv0, wv1 then wo0, wo1
                src, dst = [(wv, wvns), (wo, wons)][ci // 8]
                n = (ci // 4) % 2
                kc = 2 * (ci % 4)
                nc.sync.dma_start(dst[n][:, kc : kc + 2, :], src[n][:, kc : kc + 2, :])

            # token-major V tiles, one 512-wide half per qkv weight half
            v_tiles = [v_pool.tile([P, 2, 512], F16, tag="v", name=f"v{t}") for t in range(TC)]

            def emit_rot(pend):
                q_sb, m = pend
                rp = rot_ps.tile([P, VALID], F32, tag="rps", name=f"rp{m}")
                nc.tensor.matmul(rp[:], rot_t[:], q_sb[:], start=True, stop=True)
                t1 = tmp_pool.tile([P, VALID], F16, tag="t1", name=f"t1_{m}")
                nc.gpsimd.tensor_mul(t1[:], q_sb[:], cos_t[:])
                t2 = tmp_pool.tile([P, VALID], F16, tag="t2", name=f"t2_{m}")
                nc.vector.tensor_mul(t2[:], rp[:], sin_t[:])
                dst = qk_pool.tile([P, VALID], F16, tag="qk", name=f"qk{m}")
                if with_qkv_bias:
                    t3 = tmp_pool.tile([P, VALID], F32, tag="t3", name=f"t3_{m}")
                    nc.vector.tensor_add(t3[:], t1[:], t2[:])
                    nc.vector.tensor_add(dst[:], t3[:], qb_t[:, m, :])
                else:
                    nc.vector.tensor_add(dst[:], t1[:], t2[:])
                if m >= NH // 2:
                    k_tiles[m - NH // 2] = dst
                else:
                    qk_tiles[m] = dst

            def emit_scores(pair):
                # PE row-tiling: head 2*pair lives on contraction rows 0:64,
                # head 2*pair+1 on rows 64:128.  Both heads' scores for a key
                # chunk land in the two banks of one (128, 1024) PSUM tile --
                # the matmuls run concurrently (different row groups AND
                # banks) and a single exp covers both, halving the
                # scalar-engine per-activation overhead.  The per-key bias
                # is shared by both heads, so this is always exact.
                qt = qk_tiles[pair]
                kt = k_tiles[pair]
                for j in range(TC):
                    sp = s_ps.tile([P, 2 * VALID], F32, tag="sps",
                                   name=f"s{pair}_{j}")
                    for hh in range(2):
                        lo, hi = hh * HD, hh * HD + HD
                        nc.tensor.matmul(
                            sp[:, hh * VALID : (hh + 1) * VALID],
                            kt[lo:hi, j * P : (j + 1) * P],
                            qt[lo:hi, :],
                            start=True, stop=True,
                        )
                    ej = e_pool.tile([P, 2 * VALID], F16, tag="e",
                                     name=f"e{pair}_{j}")
                    nc.scalar.activation(
                        ej[:], sp[:], EXP, bias=bias_t[:, j : j + 1], scale=escale
                    )
                    e_tiles[(2 * pair, j)] = ej[:, 0:VALID]
                    e_tiles[(2 * pair + 1, j)] = ej[:, VALID : 2 * VALID]

            def emit_vproj(n, t):
                ps = proj_ps.tile([P, 512], F32, tag="pps", name=f"vps{n}{t}")
                for kc in range(KC):
                    nc.tensor.matmul(
                        ps[:], x_sl(kc)[:, t * P : (t + 1) * P], wvns[n][:, kc, :],
                        start=(kc == 0), stop=(kc == KC - 1),
                    )
                nc.vector.tensor_copy(v_tiles[t][:, n, :], ps[:])

            rcs = {}

            def emit_den(p):
                # softmax denominators for head pair p via a col-tiled
                # ones-matmul: each head's column sums land replicated on a
                # 64-partition half of one PSUM tile, matching the ctx^T
                # halves, so the normalize multiply reads the batched
                # reciprocal directly -- no cross-partition broadcast.
                dt_ = den_ps.tile([P, VALID], F32, tag="dps", name=f"den{p}")
                for j in range(TC):
                    for hh in range(2):
                        nc.tensor.matmul(
                            dt_[hh * HD : (hh + 1) * HD, :], ones_t[:, 0:HD],
                            e_tiles[(2 * p + hh, j)],
                            start=(j == 0), stop=(j == TC - 1),
                            skip_group_check=True,
                        )
                lg = lg_pool.tile([P, VALID], F32, tag="lg", name=f"lg{p}")
                nc.scalar.activation(lg[:], dt_[:],
                                     mybir.ActivationFunctionType.Ln)
                rc = rc_pool.tile([P, VALID], F16, tag="rc", name=f"rc{p}")
                nc.scalar.activation(rc[:], lg[:], EXP, scale=-1.0)
                rcs[p] = rc

            # ---- Phase B+C: QK proj + RoPE + scores/exp + V proj, fused ----
            # V-proj chunks fill the PE slack that row-tiled scores free
            # while the scalar engine drains the exp backlog.
            order = [m for pair in range(NH // 2) for m in (pair, NH // 2 + pair)]
            v_sched = {4: (0, 0), 5: (0, 1), 6: (0, 2), 7: (0, 3),
                       9: (1, 0), 10: (1, 1), 11: (1, 2), 12: (1, 3)}
            pend = None
            for mi, m in enumerate(order):
                if m in wm_tiles:
                    wm = wm_tiles[m]
                else:
                    wm = wqk_pool.tile([P, KC, P], F16, tag="wqk", name=f"wm{m}")
                    nc.sync.dma_start(wm[:], wqk[m])
                if 1 <= mi <= 16:
                    emit_wchunk(mi - 1)
                ps = proj_ps.tile([P, VALID], F32, tag="pps", name=f"ps{m}")
                for kc in range(KC):
                    nc.tensor.matmul(
                        ps[:], wm[:, kc, :], x_sl(kc),
                        start=(kc == 0), stop=(kc == KC - 1),
                    )
                q_sb = qsb_pool.tile([P, VALID], F16, tag="qsb", name=f"qsb{m}")
                nc.vector.tensor_copy(q_sb[:], ps[:])
                if pend is not None:
                    pm = pend[1]
                    emit_rot(pend)
                    if pm >= NH // 2:
                        emit_scores(pm - NH // 2)
                if mi in v_sched:
                    emit_vproj(*v_sched[mi])
                if mi >= 4 and mi % 2 == 0:
                    emit_den((mi - 4) // 2)
                pend = (q_sb, m)
            emit_rot(pend)
            emit_scores(NH // 2 - 1)
            emit_wchunk(15)
            emit_den(6)
            emit_den(7)

            # ---- Phase D+E: ctx + normalize, o_proj pass 0 pipelined in ----
            # ctx pairs are col-tiled (2 heads concurrently into the two
            # 64-partition halves of one PSUM bank); the reciprocals were
            # precomputed during phase B+C, so a pair's bank is freed by
            # two DVE multiplies -- no scalar/gpsimd latency in the loop.
            ctx_tiles = [ctx_pool.tile([P, VALID], F16, tag="ctx", name=f"ctx{m}") for m in range(KC)]

            # o_proj pass n=0 accumulation chains on the four single-bank
            # pools; fed column-by-column with a 3-pair stagger.
            ops0 = [
                proj_ps.tile([P, 512], F32, tag="pps", name="od00"),
                proj_ps.tile([P, 512], F32, tag="pps", name="od01"),
                rot_ps.tile([P, 512], F32, tag="rps", name="od02"),
                den_ps.tile([P, 512], F32, tag="dps", name="od03"),
            ]

            def emit_ocol(mcol, n, chains):
                won = wons[n]
                for t in range(TC):
                    nc.tensor.matmul(
                        chains[t][:], ctx_tiles[mcol][:, t * P : (t + 1) * P],
                        won[:, mcol, :],
                        start=(mcol == 0), stop=(mcol == KC - 1),
                        skip_group_check=True,
                    )

            # pairs 0-1 take proj_ps singles (free as soon as the last V
            # copies drain) so the ctx stream isn't serialized behind the
            # final scores exps by s_ps pool ring order
            cpd = None
            for p in range(KC):
                if p < 2:
                    cp = proj_ps.tile([P, VALID], F32, tag="pps", name=f"cps{p}")[:]
                else:
                    if p % 2 == 0:
                        cpd = s_ps.tile([P, 2 * VALID], F32, tag="sps", name=f"cpd{p//2}")
                    cp = cpd[:, (p % 2) * VALID : (p % 2 + 1) * VALID]
                vn, vcol = p // 4, (p % 4) * P
                for j in range(TC):
                    for hh in range(2):
                        h = 2 * p + hh
                        nc.tensor.matmul(
                            cp[hh * HD : (hh + 1) * HD, :],
                            v_tiles[j][:, vn, vcol + hh * HD : vcol + (hh + 1) * HD],
                            e_tiles[(h, j)],
                            start=(j == 0), stop=(j == TC - 1),
                            skip_group_check=True,
                        )
                for hh in range(2):
                    h = 2 * p + hh
                    dst = ctx_tiles[p][hh * HD : (hh + 1) * HD, :]
                    rch = rcs[p][hh * HD : (hh + 1) * HD, :]
                    if with_qkv_bias:
                        tmpc = tmp_pool.tile([HD, VALID], F32, tag="tc", name=f"tc{h}")
                        nc.vector.tensor_mul(tmpc[:], cp[hh * HD : (hh + 1) * HD, :], rch)
                        nc.scalar.activation(
                            dst, tmpc[:], mybir.ActivationFunctionType.Identity,
                            bias=vb_t[hh * HD : (hh + 1) * HD, p : p + 1],
                        )
                    else:
                        nc.vector.tensor_mul(dst, cp[hh * HD : (hh + 1) * HD, :], rch)
                if p >= 3:
                    emit_ocol(p - 3, 0, ops0)
            for mcol in (KC - 3, KC - 2, KC - 1):
                emit_ocol(mcol, 0, ops0)
            for t in range(TC):
                ot = o_pool.tile([P, 512], F32, tag="o", name=f"o0{t}")
                nc.vector.tensor_copy(ot[:], ops0[t][:])
                nc.sync.dma_start(out[t * P : (t + 1) * P, 0:512], ot[:])

            # o_proj pass n=1: all ctx tiles are ready; straight chains on
            # the same four single-bank pools (a shared double-tile would
            # serialize chain t+1 behind chain t's drain copy -- Tile
            # tracks write-after-read at tile granularity)
            ops1 = [
                proj_ps.tile([P, 512], F32, tag="pps", name="od10"),
                proj_ps.tile([P, 512], F32, tag="pps", name="od11"),
                rot_ps.tile([P, 512], F32, tag="rps", name="od12"),
                den_ps.tile([P, 512], F32, tag="dps", name="od13"),
            ]
            for t in range(TC):
                ps = ops1[t]
                for m in range(KC):
                    nc.tensor.matmul(
                        ps[:], ctx_tiles[m][:, t * P : (t + 1) * P], wons[1][:, m, :],
                        start=(m == 0), stop=(m == KC - 1),
                        skip_group_check=True,
                    )
                ot = o_pool.tile([P, 512], F32, tag="o", name=f"o1{t}")
                nc.vector.tensor_copy(ot[:], ps[:])
                nc.sync.dma_start(out[t * P : (t + 1) * P, 512:1024], ot[:])

    nc.compile()
    return nc


def _get_nc(with_qkv_bias):
    key = bool(with_qkv_bias)
    if key not in _CACHE:
        _CACHE[key] = _build(key)
    return _CACHE[key]


def _rot_matrix():
    # R such that (R.T @ q)[d] == rotate_half(q)[d], block-diagonal per head
    r64 = np.zeros((HD, HD), np.float32)
    half = HD // 2
    for d in range(half):
        r64[d + half, d] = -1.0  # dest d < 32 gets -q[d+32]
        r64[d, d + half] = 1.0   # dest d >= 32 gets  q[d-32]
    r = np.zeros((P, P), np.float32)
    r[:HD, :HD] = r64
    r[HD:, HD:] = r64
    return r


def _to_tiles_kxm(w, ncols):
    """(H, F) weight -> (F//ncols, P, KC, ncols) fp16, contiguous."""
    F = w.shape[1]
    t = w.reshape(KC, P, F // ncols, ncols).transpose(2, 1, 0, 3)
    return np.ascontiguousarray(t.astype(np.float16))


def kernel(hidden_states, cos, sin, attention_bias, qkv_w, qkv_b, o_w, o_b,
           indices, batch, seqlen, _trace=False):
    from concourse.bass_utils import run_bass_kernel_spmd

    hidden_states = np.asarray(hidden_states, dtype=np.float32)
    cos = np.asarray(cos, dtype=np.float32)
    sin = np.asarray(sin, dtype=np.float32)
    attention_bias = np.asarray(attention_bias, dtype=np.float32)
    qkv_w = np.asarray(qkv_w, dtype=np.float32)
    qkv_b = np.asarray(qkv_b, dtype=np.float32)
    o_w = np.asarray(o_w, dtype=np.float32)
    o_b = np.asarray(o_b, dtype=np.float32)
    indices = np.asarray(indices)
    batch = int(batch)
    seqlen = int(seqlen)

    with_bias = bool(np.any(qkv_b))

    pos = indices.astype(np.int64)
    b_of = pos // seqlen
    s_of = pos % seqlen

    wqk2 = _to_tiles_kxm(qkv_w[:, : 2 * H], P)        # (16, P, KC, P)
    wv2 = _to_tiles_kxm(qkv_w[:, 2 * H :], 512)       # (2, P, KC, 512)
    wo2 = _to_tiles_kxm(o_w, 512)                     # (2, P, KC, 512)
    rot = _rot_matrix().astype(np.float16)
    ones16 = np.ones((P, 64), np.float16)

    in_maps = []
    tok_idx = []
    for b in range(batch):
        idx = np.nonzero(b_of == b)[0]
        assert len(idx) == VALID, f"batch {b} has {len(idx)} valid tokens"
        tok_idx.append(idx)
        xT2 = np.ascontiguousarray(
            hidden_states[idx].T.reshape(KC, P, VALID).transpose(1, 0, 2)
            .astype(np.float16)
        )
        cosT = cos[idx, 0, :].T  # (HD, VALID)
        sinT = sin[idx, 0, :].T
        cos2 = np.ascontiguousarray(
            np.concatenate([cosT, cosT], axis=0).astype(np.float16))
        sin2 = np.ascontiguousarray(
            np.concatenate([sinT, sinT], axis=0).astype(np.float16))
        bias_b = attention_bias[b, 0, 0, s_of[idx]].astype(np.float32)  # (VALID,)
        biask = np.ascontiguousarray(bias_b.reshape(TC, P).T)  # (P, TC)
        m = {
            "xT": xT2, "wqk": wqk2, "wv": wv2, "wo": wo2,
            "cos2": cos2, "sin2": sin2, "rot": rot, "biask": biask,
            "ones16": ones16,
        }
        if with_bias:
            bq = qkv_b[: 2 * H]
            cos_full = np.tile(cosT, (2 * H // HD, 1))  # (2H, VALID)
            sin_full = np.tile(sinT, (2 * H // HD, 1))
            rot_bq = bq.reshape(-1, 2, HD // 2)[:, ::-1, :].copy()
            rot_bq[:, 0, :] *= -1.0
            rot_bq = rot_bq.reshape(-1)
            qb = (bq[:, None] * cos_full + rot_bq[:, None] * sin_full)
            qb = qb.reshape(QK_TILES, P, VALID).transpose(1, 0, 2)
            m["qb_rope"] = np.ascontiguousarray(qb.astype(np.float32))
            bv = qkv_b[2 * H :].astype(np.float32)
            m["vbias"] = np.ascontiguousarray(bv.reshape(KC, P).T)
        in_maps.append(m)

    nc = _get_nc(with_bias)
    res = run_bass_kernel_spmd(nc, in_maps, core_ids=list(range(B)), trace=_trace)

    T = hidden_states.shape[0]
    out_full = np.empty((T, H), np.float32)
    for b in range(batch):
        out_full[tok_idx[b]] = res.results[b]["out"]
    if np.any(o_b):
        out_full += o_b[None, :]
    if _trace:
        kernel.last_exec_time_ns = res.exec_time_ns
        kernel.last_results = res
    return out_full
